# revision 4
# baseline (speedup 1.0000x reference)
"""MoE (top-2 of 8 experts) Trainium2 kernel, 8-core data-parallel over tokens.

Problem shapes (hardcoded): x [4, 2048, 512] f32, Wg [512, 8], W1 [8, 512, 1024],
b1 [8, 1024], W2 [8, 1024, 512], b2 [8, 512].  T = 8192 tokens, top-2 routing.

Strategy: shard tokens across the 8 cores (1024 tokens/core); replicate the
router and all expert weights (weights cast to bf16 host-side, halving HBM
traffic).  Each core computes, fully on device:
  - xT (PE transpose), fp32 router logits -> softmax -> top-2 gates
  - per expert: hT = gelu_tanh(x @ W1_e + b1_e) (bf16 matmuls, f32 psum),
    y = hT.T @ W2_e, out += gate_e * y
The reference itself is a dense masked-gate MoE, so the dense per-expert loop
is numerically equivalent.  No cross-core communication is needed.
"""

from contextlib import ExitStack

import numpy as np
import ml_dtypes

import concourse.bass as bass
import concourse.tile as tile
from concourse import bacc, mybir
from concourse.bass_utils import run_bass_kernel_spmd
from concourse.masks import make_identity

P = 128
N_CORES = 8
B, S, D, H, O, E = 4, 2048, 512, 1024, 512, 8
T = B * S                    # 8192
TC = T // N_CORES            # 1024 tokens per core
DC = D // P                  # 4 D-chunks
HC = H // P                  # 8 H-chunks
NT = TC // P                 # 8 token tiles of 128
TW = 512                     # GEMM1 moving-dim width
NTW = TC // TW               # 2

MM_DT = mybir.dt.bfloat16
NP_MM_DT = ml_dtypes.bfloat16
F32 = mybir.dt.float32
AF = mybir.ActivationFunctionType


def build_nc(has_b1: bool, has_b2: bool) -> bass.Bass:
    nc = bacc.Bacc()
    x_d = nc.declare_dram_parameter("x", [TC, D], F32, isOutput=False)
    wg_d = nc.declare_dram_parameter("wg", [D, E], F32, isOutput=False)
    w1_d = nc.declare_dram_parameter("w1", [E, D, H], MM_DT, isOutput=False)
    w2_d = nc.declare_dram_parameter("w2", [E, H, O], MM_DT, isOutput=False)
    if has_b1:
        b1_d = nc.declare_dram_parameter("b1", [E, H], F32, isOutput=False)
    if has_b2:
        b2_d = nc.declare_dram_parameter("b2", [E, O], F32, isOutput=False)
    out_d = nc.declare_dram_parameter("out", [TC, O], F32, isOutput=True)

    with ExitStack() as ctx:
        tc = ctx.enter_context(tile.TileContext(nc))
        singles = ctx.enter_context(tc.tile_pool(name="singles", bufs=1))
        xload = ctx.enter_context(tc.tile_pool(name="xload", bufs=3))
        wpool = ctx.enter_context(tc.tile_pool(name="wpool", bufs=2))
        hpool = ctx.enter_context(tc.tile_pool(name="hpool", bufs=2))
        tmp = ctx.enter_context(tc.tile_pool(name="tmp", bufs=4))
        psum_t = ctx.enter_context(tc.tile_pool(name="psum_t", bufs=2, space="PSUM"))
        psum_r = ctx.enter_context(tc.tile_pool(name="psum_r", bufs=2, space="PSUM"))
        psum_h = ctx.enter_context(tc.tile_pool(name="psum_h", bufs=2, space="PSUM"))
        psum_y = ctx.enter_context(tc.tile_pool(name="psum_y", bufs=2, space="PSUM"))

        ident = singles.tile([P, P], F32)
        make_identity(nc, ident)

        wg_sb = singles.tile([P, DC, E], F32)
        nc.sync.dma_start(wg_sb, wg_d[:].rearrange("(c p) e -> p c e", p=P))
        if has_b1:
            b1_sb = singles.tile([P, HC, E], F32)
            with nc.allow_non_contiguous_dma(reason="tiny one-time b1 load"):
                nc.sync.dma_start(b1_sb, b1_d[:].rearrange("e (c p) -> p c e", p=P))
        if has_b2:
            # broadcast b2[e] across partitions: [P, E, O] with partition stride 0
            b2_sb = singles.tile([P, E, O], F32)
            b2_ap = b2_d[:]
            b2_bcast = bass.AP(
                tensor=b2_ap.tensor,
                offset=b2_ap.offset,
                ap=[[0, P], *b2_ap.ap],
            )
            nc.sync.dma_start(b2_sb, b2_bcast)

        xT32 = singles.tile([P, DC, TC], F32)
        xT16 = singles.tile([P, DC, TC], MM_DT)
        gates = singles.tile([P, NT, E], F32)
        out_acc = singles.tile([P, NT, O], F32)

        # ---- transpose x into xT (fp32 for router, bf16 for expert GEMMs) ----
        for tt in range(NT):
            xr = xload.tile([P, D], F32)
            nc.sync.dma_start(xr, x_d[:][tt * P:(tt + 1) * P, :])
            for dc in range(DC):
                pt = psum_t.tile([P, P], F32)
                nc.tensor.transpose(pt, xr[:, dc * P:(dc + 1) * P], ident)
                nc.vector.tensor_copy(xT32[:, dc, tt * P:(tt + 1) * P], pt)
                nc.scalar.copy(xT16[:, dc, tt * P:(tt + 1) * P], pt)

        # ---- router: logits -> softmax -> top-2 gates (dense, zero-masked) ----
        for tt in range(NT):
            pr = psum_r.tile([P, E], F32)
            for dc in range(DC):
                nc.tensor.matmul(
                    pr,
                    lhsT=xT32[:, dc, tt * P:(tt + 1) * P],
                    rhs=wg_sb[:, dc, :],
                    start=(dc == 0),
                    stop=(dc == DC - 1),
                )
            ex = tmp.tile([P, E], F32, tag="ex")
            s = tmp.tile([P, 1], F32, tag="s")
            nc.scalar.activation(out=ex, in_=pr, func=AF.Exp, accum_out=s)
            rec = tmp.tile([P, 1], F32, tag="rec")
            nc.vector.reciprocal(rec, s)
            nc.vector.tensor_scalar_mul(ex, ex, rec)  # probs
            top8 = tmp.tile([P, 8], F32, tag="top8")
            nc.vector.max(out=top8, in_=ex)
            mask = tmp.tile([P, E], F32, tag="mask")
            nc.vector.tensor_scalar(
                out=mask, in0=ex, scalar1=top8[:, 1:2], scalar2=None,
                op0=mybir.AluOpType.is_ge,
            )
            nc.vector.tensor_tensor(
                out=gates[:, tt, :], in0=ex, in1=mask, op=mybir.AluOpType.mult
            )

        # ---- experts ----
        for e in range(E):
            w1_sb = wpool.tile([P, DC, H], MM_DT, tag="w1")
            nc.sync.dma_start(w1_sb, w1_d[:][e].rearrange("(c p) h -> p c h", p=P))
            w2_sb = wpool.tile([P, HC, O], MM_DT, tag="w2")
            nc.sync.dma_start(w2_sb, w2_d[:][e].rearrange("(c p) o -> p c o", p=P))

            for tw in range(NTW):
                h_sb = hpool.tile([P, HC, TW], MM_DT, tag="h")
                for hc in range(HC):
                    ph = psum_h.tile([P, TW], F32)
                    for dc in range(DC):
                        nc.tensor.matmul(
                            ph,
                            lhsT=w1_sb[:, dc, hc * P:(hc + 1) * P],
                            rhs=xT16[:, dc, tw * TW:(tw + 1) * TW],
                            start=(dc == 0),
                            stop=(dc == DC - 1),
                        )
                    bias_ap = b1_sb[:, hc, e:e + 1] if has_b1 else 0.0
                    nc.scalar.activation(
                        out=h_sb[:, hc, :], in_=ph, func=AF.Gelu_apprx_tanh,
                        bias=bias_ap,
                    )
                for ts4 in range(TW // P):
                    tt = tw * (TW // P) + ts4
                    py = psum_y.tile([P, O], F32)
                    for hc in range(HC):
                        nc.tensor.matmul(
                            py,
                            lhsT=h_sb[:, hc, ts4 * P:(ts4 + 1) * P],
                            rhs=w2_sb[:, hc, :],
                            start=(hc == 0),
                            stop=(hc == HC - 1),
                        )
                    g_ap = gates[:, tt, e:e + 1]
                    if has_b2:
                        yb = tmp.tile([P, O], F32, tag="yb")
                        nc.vector.tensor_add(yb, py, b2_sb[:, e, :])
                        src = yb
                    else:
                        src = py
                    if e == 0:
                        nc.vector.tensor_scalar_mul(
                            out=out_acc[:, tt, :], in0=src, scalar1=g_ap
                        )
                    else:
                        yt = tmp.tile([P, O], F32, tag="yt")
                        nc.vector.tensor_scalar_mul(out=yt, in0=src, scalar1=g_ap)
                        nc.vector.tensor_add(
                            out_acc[:, tt, :], out_acc[:, tt, :], yt
                        )

        nc.sync.dma_start(out_d[:].rearrange("(t p) o -> p t o", p=P), out_acc)

    nc.finalize()
    return nc


_NC_CACHE: dict = {}


def _get_nc(has_b1: bool, has_b2: bool) -> bass.Bass:
    key = (has_b1, has_b2)
    if key not in _NC_CACHE:
        _NC_CACHE[key] = build_nc(has_b1, has_b2)
    return _NC_CACHE[key]


def kernel(x, Wg, W1, b1, W2, b2, _trace=False, _tmpdir=None):
    x = np.ascontiguousarray(np.asarray(x, dtype=np.float32))
    Wg = np.ascontiguousarray(np.asarray(Wg, dtype=np.float32))
    W1 = np.asarray(W1, dtype=np.float32)
    b1 = np.asarray(b1, dtype=np.float32)
    W2 = np.asarray(W2, dtype=np.float32)
    b2 = np.asarray(b2, dtype=np.float32)

    has_b1 = bool(np.any(b1))
    has_b2 = bool(np.any(b2))
    nc = _get_nc(has_b1, has_b2)

    xm = x.reshape(T, D)
    w1_bf = np.ascontiguousarray(W1.astype(NP_MM_DT))
    w2_bf = np.ascontiguousarray(W2.astype(NP_MM_DT))

    base = {"wg": Wg, "w1": w1_bf, "w2": w2_bf}
    if has_b1:
        base["b1"] = np.ascontiguousarray(b1)
    if has_b2:
        base["b2"] = np.ascontiguousarray(b2)

    in_maps = [
        {**base, "x": np.ascontiguousarray(xm[c * TC:(c + 1) * TC])}
        for c in range(N_CORES)
    ]
    res = run_bass_kernel_spmd(
        nc, in_maps, core_ids=list(range(N_CORES)), trace=_trace, tmpdir=_tmpdir
    )
    out = np.concatenate([res.results[c]["out"] for c in range(N_CORES)], axis=0)
    if _trace:
        kernel._last_result = res
    return out.reshape(B, S, O).astype(np.float32)
